# revision 4
# baseline (speedup 1.0000x reference)
"""Fused 2D-RoPE multi-head attention block for Trainium2, SPMD over 8 NeuronCores.

Problem: x[2,4,24,24,1024] -> qkv proj -> 16-head attention with 2-axis RoPE
-> out proj.  Data-parallel: one (b t) sequence (S=576 tokens, D=1024) per core.

Key device-side choices (v2):
  - ALL weight/x tensors are host-packed into flat [128, N] layouts so every
    DMA is one contiguous descriptor per partition (the baseline's 512B-line
    weight DMAs measured 2.4x slower than packed DMAs).
  - RoPE rotate-half is ONE stream_shuffle: q/k weight rows are host-permuted
    so each 32-partition quadrant holds [16 even-slots | 16 odd-slots] and the
    pair-swap is the fixed mask [16..31,0..15].  Per projection half:
    shuffle (DVE), two muls (Pool), add (DVE) - all reading qk PSUM directly,
    so the Activation engine runs ONLY the 160 softmax exps per rep.
  - scoresT = k-stationary x q-moving per head (64-partition contraction),
    exp on Act (scale=0.125, no max subtraction: scores are well-bounded),
    att@v via v-augmented-with-ones stationary so the softmax denominator
    falls out of the same accumulation; normalize via DVE recip + Pool
    broadcast + tensor muls split across DVE/Pool.
  - roped q/k stored bf16 (halves SBUF; scores matmuls run bf16 at the same
    1 cycle/row rate); everything else float32r with 256/288-wide moving
    chunks (fp32r is full rate at free size >= 256).
  - v/out projections run st-outer (one live PSUM accumulation tile) to fit
    the attention + projection working set in 8 PSUM banks.
  - SOFTWARE PIPELINE: attention+out-proj of rep r-1 is interleaved
    unit-by-unit with x-load/v-proj/qk-proj of rep r, so the Act-paced
    attention phase hides under projection matmuls and the PE stream never
    starves.  x and v buffers are double-buffered; roped/oT single (the
    interleave order staggers their reuse safely).
"""

import numpy as np
from contextlib import ExitStack


def _bf16_np():
    import ml_dtypes
    return ml_dtypes.bfloat16

B, T, HH, WW, D = 2, 4, 24, 24, 1024
NH, HD = 16, 64
S = HH * WW            # 576
BT = B * T             # 8
NCORES = 8
P = 128
SQH = 288              # half of S; moving-dim per scores/att@v matmul
NKD = D // P           # 8 contraction tiles over D
S_TILES = [(0, 128), (128, 128), (256, 128), (384, 128), (512, 64)]
VSLOT = HD + 1         # 65: per-head v columns + ones column
SHUF_MASK = list(range(16, 32)) + list(range(16))

_CACHE: dict = {}


def _head_perm():
    """Per-head row order: [evens 0..30, odds 1..31, evens 32..62, odds 33..63]
    so the RoPE pair-partner sits +-16 partitions away inside one 32-quadrant."""
    p64 = np.concatenate([np.arange(0, 32, 2), np.arange(1, 32, 2),
                          np.arange(32, 64, 2), np.arange(33, 64, 2)])
    return (np.arange(NH)[:, None] * HD + p64[None, :]).reshape(-1)     # [1024]


def _rope_tables():
    """cos/sin tables [128, S] matching the per-head row permutation; sin rows
    for even-slots are pre-negated so roped = ps*cos + shuffle(ps)*sin."""
    half = HD // 4     # 16
    inv = (1.0 / (10000.0 ** (np.arange(half, dtype=np.float32) / np.float32(half)))).astype(np.float32)
    th = np.arange(HH, dtype=np.float32)[:, None] * inv[None, :]
    tw = np.arange(WW, dtype=np.float32)[:, None] * inv[None, :]
    cosg = np.concatenate([
        np.broadcast_to(np.cos(th)[:, None, :], (HH, WW, half)),
        np.broadcast_to(np.cos(tw)[None, :, :], (HH, WW, half))], axis=-1).reshape(S, 2 * half)
    sing = np.concatenate([
        np.broadcast_to(np.sin(th)[:, None, :], (HH, WW, half)),
        np.broadcast_to(np.sin(tw)[None, :, :], (HH, WW, half))], axis=-1).reshape(S, 2 * half)
    r = np.arange(HD)
    j = (r % 16) + 16 * (r // 32)
    sign = np.where((r % 32) < 16, -1.0, 1.0).astype(np.float32)
    cos64 = cosg.T[j]                          # [64, S]
    sin64 = sing.T[j] * sign[:, None]
    cosb = np.ascontiguousarray(np.vstack([cos64, cos64]).astype(np.float32))
    sinb = np.ascontiguousarray(np.vstack([sin64, sin64]).astype(np.float32))
    return cosb, sinb


def _build_nc(repeat=1):
    import concourse.bacc as bacc
    import concourse.mybir as mybir
    from concourse.tile import TileContext

    f32 = mybir.dt.float32
    f32r = mybir.dt.float32r
    bf16 = mybir.dt.bfloat16
    AF = mybir.ActivationFunctionType

    nc = bacc.Bacc("TRN2", target_bir_lowering=False, debug=False)
    x_d = nc.dram_tensor("xp", [P, NKD * S], bf16, kind="ExternalInput").ap()
    wqk_d = nc.dram_tensor("wqkp", [P, 16 * NKD * P], bf16, kind="ExternalInput").ap()
    wv_d = nc.dram_tensor("wvp", [P, 4 * NKD * 256], bf16, kind="ExternalInput").ap()
    wo_d = nc.dram_tensor("wop", [P, 4 * NKD * 256], bf16, kind="ExternalInput").ap()
    cos_d = nc.dram_tensor("cosb", [P, S], f32, kind="ExternalInput").ap()
    sin_d = nc.dram_tensor("sinb", [P, S], f32, kind="ExternalInput").ap()
    ones_d = nc.dram_tensor("onesc", [P, 5 * NH], bf16, kind="ExternalInput").ap()
    out_d = nc.dram_tensor("out", [S, D], f32, kind="ExternalOutput").ap()

    with TileContext(nc) as tc, ExitStack() as ctx:
        const = ctx.enter_context(tc.tile_pool(name="const", bufs=1))
        wqkp = ctx.enter_context(tc.tile_pool(name="wqkp", bufs=3))
        wvp = ctx.enter_context(tc.tile_pool(name="wvp", bufs=2))
        wop = ctx.enter_context(tc.tile_pool(name="wop", bufs=5))
        swp = ctx.enter_context(tc.tile_pool(name="swp", bufs=4))
        m2p = ctx.enter_context(tc.tile_pool(name="m2p", bufs=4))
        ep = ctx.enter_context(tc.tile_pool(name="ep", bufs=44))
        r1p = ctx.enter_context(tc.tile_pool(name="r1p", bufs=4))
        rrp = ctx.enter_context(tc.tile_pool(name="rrp", bufs=4))
        stp = ctx.enter_context(tc.tile_pool(name="stp", bufs=2))
        pschain = ctx.enter_context(tc.tile_pool(name="pschain", bufs=3, space="PSUM"))
        pssc = ctx.enter_context(tc.tile_pool(name="pssc", bufs=3, space="PSUM"))
        psav = ctx.enter_context(tc.tile_pool(name="psav", bufs=2, space="PSUM"))

        # ---- resident tensors
        cosb = const.tile([P, S], f32, name="cosb_t")
        sinb = const.tile([P, S], f32, name="sinb_t")
        xt2 = [const.tile([P, NKD * S], bf16, name=f"xt{i}") for i in range(2)]
        roped = const.tile([P, 2 * NH * S], bf16, name="roped")
        va2 = [const.tile([P, 5 * NH * VSLOT], bf16, name=f"va{i}") for i in range(2)]
        vav2 = [va.rearrange("p (j h c) -> p j h c", j=5, c=VSLOT) for va in va2]
        oT = const.tile([P, NKD * S], bf16, name="oT")

        nc.sync.dma_start(cosb[:, :], cos_d[:, :])
        nc.sync.dma_start(sinb[:, :], sin_d[:, :])
        for va in va2:
            nc.sync.dma_start(va.rearrange("p (g c) -> p g c", c=VSLOT)[:, :, HD:HD + 1],
                              ones_d[:, :])

        # weight tiles keyed by (rep, idx) so prefetch and use sites agree
        wqk_tiles: dict = {}
        wv_tiles: dict = {}
        wo_tiles: dict = {}

        def dma_x(r):
            nc.sync.dma_start(xt2[r % 2][:, :], x_d[:, :])

        def dma_wqk(r, g):
            # one DMA covers e-tiles 2g and 2g+1
            wt = wqkp.tile([P, 2 * NKD * P], bf16, name="wt")
            wqk_tiles[(r, g)] = wt
            nc.sync.dma_start(wt[:, :], wqk_d[:, g * 2 * NKD * P:(g + 1) * 2 * NKD * P])

        def dma_wv(r, g):
            wt = wvp.tile([P, NKD * 256], bf16, name="wvt")
            wv_tiles[(r, g)] = wt
            nc.sync.dma_start(wt[:, :], wv_d[:, g * NKD * 256:(g + 1) * NKD * 256])

        def dma_wo(r, g):
            wt = wop.tile([P, NKD * 256], bf16, name="wot")
            wo_tiles[(r, g)] = wt
            nc.sync.dma_start(wt[:, :], wo_d[:, g * NKD * 256:(g + 1) * NKD * 256])

        # ---- EARLY units: v-proj chains and qk-proj + rope -------------------
        def unit_v(r, g, st):
            xt = xt2[r % 2]
            vav = vav2[r % 2]
            s0, sl = S_TILES[st]
            if st == 4 and g < 2:          # prefetch this rep's later v weights
                dma_wv(r, g + 2)
            wvt = wv_tiles[(r, g)]
            ps = pschain.tile([P, 256], f32, tag="pc", name="psv")
            for kt in range(NKD):
                nc.tensor.matmul(ps[0:sl, :], xt[:, kt * S + s0:kt * S + s0 + sl],
                                 wvt[:, kt * 256:(kt + 1) * 256],
                                 start=(kt == 0), stop=(kt == NKD - 1))
            dst = vav[0:sl, st:st + 1, g * 4:(g + 1) * 4, 0:HD]
            nc.scalar.activation(dst, ps[0:sl, :].rearrange("p (h cc) -> p h cc", cc=HD),
                                 AF.Copy)

        def unit_qk(r, et, half):
            xt = xt2[r % 2]
            if half == 0:
                if et % 2 == 1 and 2 <= (et + 3) // 2 <= 7:
                    dma_wqk(r, (et + 3) // 2)   # group g at QK(2g-3,0)
                # next-rep prefetches attached at fixed spots in the qk stream
                if r + 1 < repeat:
                    if et == 8:
                        dma_wv(r + 1, 0)
                    elif et == 10:
                        dma_wv(r + 1, 1)
                    elif et == 11:
                        dma_x(r + 1)
                    elif et == 13:
                        dma_wqk(r + 1, 0)
                    elif et == 15:
                        dma_wqk(r + 1, 1)
            wt = wqk_tiles[(r, et // 2)]
            eo = et % 2
            ps = pschain.tile([P, SQH], f32, tag="pc", name="psqk")
            c0 = half * SQH
            for kt in range(NKD):
                nc.tensor.matmul(ps[:, :], wt[:, (eo * NKD + kt) * P:(eo * NKD + kt + 1) * P],
                                 xt[:, kt * S + c0:kt * S + c0 + SQH],
                                 start=(kt == 0), stop=(kt == NKD - 1))
            sw = swp.tile([P, SQH], f32, name="sw")
            nc.vector.stream_shuffle(sw[:, :], ps[:, :], SHUF_MASK)
            m2 = m2p.tile([P, 2 * SQH], f32, name="m2")
            nc.gpsimd.tensor_mul(m2[:, 0:SQH], sw[:, :], sinb[:, c0:c0 + SQH])
            nc.vector.tensor_mul(m2[:, SQH:2 * SQH], ps[:, :], cosb[:, c0:c0 + SQH])
            nc.gpsimd.tensor_add(roped[:, et * S + c0:et * S + c0 + SQH],
                                 m2[:, 0:SQH], m2[:, SQH:2 * SQH])

        # ---- LATE units: scores+exp, att@v+normalize, out-proj ---------------
        def unit_sc(r, ti, hf, jg):
            if hf == 0 and jg == 0 and ti >= 3 and ti <= 6:
                dma_wo(r, ti - 3)
            qb = ti * S
            kb = (8 + ti) * S
            col = slice(qb + hf * SQH, qb + (hf + 1) * SQH)
            js = [(0, 1), (2, 3), (4,)][jg]
            out = []
            for j in js:
                k0, kl = S_TILES[j]
                for sub in range(2):
                    off = sub * 64
                    ps = pssc.tile([P, SQH], f32, tag="sc", name="ps_s")
                    nc.tensor.matmul(ps[0:kl, :],
                                     roped[off:off + 64, kb + k0:kb + k0 + kl],
                                     roped[off:off + 64, col],
                                     start=True, stop=True)
                    E = ep.tile([P, SQH], bf16, name="E")
                    nc.scalar.activation(E[0:kl, :], ps[0:kl, :], AF.Exp, scale=0.125)
                    out.append((j, sub, E))
            return out

        def unit_av(r, ti, hf, Es):
            vav = vav2[r % 2]
            for sub in range(2):
                h = 2 * ti + sub
                off = sub * 64
                ps_o = psav.tile([P, SQH], f32, tag="av", name="ps_o")
                for j, (k0, kl) in enumerate(S_TILES):
                    nc.tensor.matmul(ps_o[0:VSLOT, :],
                                     vav[0:kl, j:j + 1, h:h + 1, :],
                                     Es[(ti, hf, j, sub)][0:kl, :],
                                     start=(j == 0), stop=(j == 4))
                r1 = r1p.tile([1, SQH], f32, name="r1")
                nc.vector.reciprocal(r1[:, :], ps_o[HD:HD + 1, :])
                rr = rrp.tile([64, SQH], f32, name="rr")
                nc.gpsimd.partition_broadcast(rr[:, :], r1[:, :])
                dst = oT[off:off + 64, ti * S + hf * SQH:ti * S + (hf + 1) * SQH]
                nc.vector.tensor_mul(dst, ps_o[0:HD, :], rr[:, :])

        def unit_op(r, st):
            s0, sl = S_TILES[st]
            stg = stp.tile([P, D], f32, name="stg")
            for g in range(4):
                wot = wo_tiles[(r, g)]
                ps = pschain.tile([P, 256], f32, tag="pc", name="pso")
                for kt in range(NKD):
                    nc.tensor.matmul(ps[0:sl, :], oT[:, kt * S + s0:kt * S + s0 + sl],
                                     wot[:, kt * 256:(kt + 1) * 256],
                                     start=(kt == 0), stop=(kt == NKD - 1))
                nc.vector.tensor_copy(stg[0:sl, g * 256:(g + 1) * 256], ps[0:sl, :])
            nc.sync.dma_start(out_d[s0:s0 + sl, :], stg[0:sl, :])

        # ---- emission schedule ----------------------------------------------
        def early_units(r):
            units = []
            for g in range(4):
                for st in range(5):
                    units.append(lambda r=r, g=g, st=st: unit_v(r, g, st))
            for et in range(16):
                for half in range(2):
                    units.append(lambda r=r, et=et, half=half: unit_qk(r, et, half))
            return units

        def late_units(r):
            Es: dict = {}

            def sc(ti, hf, jg):
                for j, sub, E in unit_sc(r, ti, hf, jg):
                    Es[(ti, hf, j, sub)] = E

            def sc_block(ti):
                return [lambda ti=ti, hf=hf, jg=jg: sc(ti, hf, jg)
                        for hf in range(2) for jg in range(3)]

            # Es is mutated as sc units run; av reads it lazily at call time.
            def av_unit(ti, hf):
                return lambda: unit_av(r, ti, hf, Es)

            # SC(0), SC(1), AV(0), SC(2), AV(1), ..., SC(7), AV(6), AV(7)
            units = []
            units += sc_block(0)
            units += sc_block(1)
            units += [av_unit(0, 0), av_unit(0, 1)]
            for ti in range(2, 8):
                units += sc_block(ti)
                units += [av_unit(ti - 1, 0), av_unit(ti - 1, 1)]
            units += [av_unit(7, 0), av_unit(7, 1)]
            for st in range(5):
                units.append(lambda st=st: unit_op(r, st))
            return units

        def merged_emit(late, early):
            nl, ne = len(late), len(early)
            li = ei = 0
            while li < nl or ei < ne:
                if li < nl and (ei >= ne or li * ne <= ei * nl):
                    late[li]()
                    li += 1
                else:
                    early[ei]()
                    ei += 1

        # prologue DMAs for rep 0
        dma_x(0)
        dma_wv(0, 0)
        dma_wv(0, 1)
        dma_wqk(0, 0)
        dma_wqk(0, 1)

        for r in range(repeat):
            merged_emit(late_units(r - 1) if r > 0 else [], early_units(r))
        merged_emit(late_units(repeat - 1), [])

    nc.compile()
    return nc


def _prep_inputs(x, w_qkv, w_out):
    x = np.asarray(x, dtype=np.float32)
    w_qkv = np.asarray(w_qkv, dtype=np.float32)
    w_out = np.asarray(w_out, dtype=np.float32)
    xr = x.reshape(BT, S, D)
    perm = _head_perm()
    wq = w_qkv[0:D][perm]
    wk = w_qkv[D:2 * D][perm]
    wqkT = np.concatenate([wq, wk], axis=0).T          # [1024 d, 2048 e]
    # pack: [d=kt*128+p, e=et*128+c] -> [p, et*1024 + kt*128 + c]
    wqk_pack = np.ascontiguousarray(
        wqkT.reshape(NKD, P, 16, P).transpose(1, 2, 0, 3).reshape(P, 16 * NKD * P)
        .astype(_bf16_np()))
    wvT = w_qkv[2 * D:3 * D].T                         # [1024 d, 1024 e]
    wv_pack = np.ascontiguousarray(
        wvT.reshape(NKD, P, 4, 256).transpose(1, 2, 0, 3).reshape(P, 4 * NKD * 256)
        .astype(_bf16_np()))
    woT = w_out.T
    wo_pack = np.ascontiguousarray(
        woT.reshape(NKD, P, 4, 256).transpose(1, 2, 0, 3).reshape(P, 4 * NKD * 256)
        .astype(_bf16_np()))
    cosb, sinb = _rope_tables()
    in_maps = []
    for i in range(NCORES):
        xp = np.ascontiguousarray(
            xr[i].T.reshape(NKD, P, S).transpose(1, 0, 2).reshape(P, NKD * S)
            .astype(_bf16_np()))
        in_maps.append({
            "xp": xp,
            "wqkp": wqk_pack, "wvp": wv_pack, "wop": wo_pack,
            "cosb": cosb, "sinb": sinb,
            "onesc": np.ones((P, 5 * NH), dtype=_bf16_np()),
        })
    return in_maps


def get_nc(repeat=1):
    key = f"nc{repeat}"
    if key not in _CACHE:
        _CACHE[key] = _build_nc(repeat)
    return _CACHE[key]


def kernel(x, w_qkv, w_out, b_out):
    from concourse import bass_utils
    nc = get_nc()
    in_maps = _prep_inputs(x, w_qkv, w_out)
    res = bass_utils.run_bass_kernel_spmd(nc, in_maps, core_ids=list(range(NCORES)))
    out = np.stack([res.results[i]["out"] for i in range(NCORES)], axis=0)
    out = out + np.asarray(b_out, dtype=np.float32)[None, None, :]
    return np.ascontiguousarray(out.reshape(B, T, HH, WW, D).astype(np.float32))


# revision 7
# speedup vs baseline: 1.2437x; 1.2437x over previous
"""Fused 2D-RoPE multi-head attention block for Trainium2, SPMD over 8 NeuronCores.

Problem: x[2,4,24,24,1024] -> qkv proj -> 16-head attention with 2-axis RoPE
-> out proj.  Data-parallel: one (b t) sequence (S=576 tokens, D=1024) per core.

Key device-side choices (v2):
  - ALL weight/x tensors are host-packed into flat [128, N] layouts so every
    DMA is one contiguous descriptor per partition (the baseline's 512B-line
    weight DMAs measured 2.4x slower than packed DMAs).
  - RoPE rotate-half is ONE stream_shuffle: q/k weight rows are host-permuted
    so each 32-partition quadrant holds [16 even-slots | 16 odd-slots] and the
    pair-swap is the fixed mask [16..31,0..15].  Per projection half:
    shuffle (DVE), two muls (Pool), add (DVE) - all reading qk PSUM directly,
    so the Activation engine runs ONLY the 160 softmax exps per rep.
  - scoresT = k-stationary x q-moving per head (64-partition contraction),
    exp on Act (scale=0.125, no max subtraction: scores are well-bounded),
    att@v via v-augmented-with-ones stationary so the softmax denominator
    falls out of the same accumulation; normalize via DVE recip + Pool
    broadcast + tensor muls split across DVE/Pool.
  - roped q/k stored bf16 (halves SBUF; scores matmuls run bf16 at the same
    1 cycle/row rate); everything else float32r with 256/288-wide moving
    chunks (fp32r is full rate at free size >= 256).
  - v/out projections run st-outer (one live PSUM accumulation tile) to fit
    the attention + projection working set in 8 PSUM banks.
  - SOFTWARE PIPELINE: attention+out-proj of rep r-1 is interleaved
    unit-by-unit with x-load/v-proj/qk-proj of rep r, so the Act-paced
    attention phase hides under projection matmuls and the PE stream never
    starves.  x and v buffers are double-buffered; roped/oT single (the
    interleave order staggers their reuse safely).
"""

import numpy as np
from contextlib import ExitStack


def _bf16_np():
    import ml_dtypes
    return ml_dtypes.bfloat16

B, T, HH, WW, D = 2, 4, 24, 24, 1024
NH, HD = 16, 64
S = HH * WW            # 576
BT = B * T             # 8
NCORES = 8
P = 128
SQH = 288              # half of S; moving-dim per scores/att@v matmul
NKD = D // P           # 8 contraction tiles over D
S_TILES = [(0, 128), (128, 128), (256, 128), (384, 128), (512, 64)]
VSLOT = HD + 1         # 65: per-head v columns + ones column
SHUF_MASK = list(range(16, 32)) + list(range(16))

_CACHE: dict = {}


def _head_perm():
    """Per-head row order: [evens 0..30, odds 1..31, evens 32..62, odds 33..63]
    so the RoPE pair-partner sits +-16 partitions away inside one 32-quadrant."""
    p64 = np.concatenate([np.arange(0, 32, 2), np.arange(1, 32, 2),
                          np.arange(32, 64, 2), np.arange(33, 64, 2)])
    return (np.arange(NH)[:, None] * HD + p64[None, :]).reshape(-1)     # [1024]


def _rope_tables():
    """cos/sin tables [128, S] matching the per-head row permutation; sin rows
    for even-slots are pre-negated so roped = ps*cos + shuffle(ps)*sin."""
    half = HD // 4     # 16
    inv = (1.0 / (10000.0 ** (np.arange(half, dtype=np.float32) / np.float32(half)))).astype(np.float32)
    th = np.arange(HH, dtype=np.float32)[:, None] * inv[None, :]
    tw = np.arange(WW, dtype=np.float32)[:, None] * inv[None, :]
    cosg = np.concatenate([
        np.broadcast_to(np.cos(th)[:, None, :], (HH, WW, half)),
        np.broadcast_to(np.cos(tw)[None, :, :], (HH, WW, half))], axis=-1).reshape(S, 2 * half)
    sing = np.concatenate([
        np.broadcast_to(np.sin(th)[:, None, :], (HH, WW, half)),
        np.broadcast_to(np.sin(tw)[None, :, :], (HH, WW, half))], axis=-1).reshape(S, 2 * half)
    r = np.arange(HD)
    j = (r % 16) + 16 * (r // 32)
    sign = np.where((r % 32) < 16, -1.0, 1.0).astype(np.float32)
    cos64 = cosg.T[j]                          # [64, S]
    sin64 = sing.T[j] * sign[:, None]
    cosb = np.ascontiguousarray(np.vstack([cos64, cos64]).astype(np.float32))
    sinb = np.ascontiguousarray(np.vstack([sin64, sin64]).astype(np.float32))
    return cosb, sinb


def _build_nc(repeat=1):
    import concourse.bacc as bacc
    import concourse.mybir as mybir
    from concourse.tile import TileContext

    f32 = mybir.dt.float32
    f32r = mybir.dt.float32r
    bf16 = mybir.dt.bfloat16
    AF = mybir.ActivationFunctionType

    nc = bacc.Bacc("TRN2", target_bir_lowering=False, debug=False)
    x_d = nc.dram_tensor("xp", [P, NKD * S], bf16, kind="ExternalInput").ap()
    wqk_d = nc.dram_tensor("wqkp", [P, 16 * NKD * P], bf16, kind="ExternalInput").ap()
    wv_d = nc.dram_tensor("wvp", [P, 2 * NKD * 512], bf16, kind="ExternalInput").ap()
    wo_d = nc.dram_tensor("wop", [P, 2 * NKD * 512], bf16, kind="ExternalInput").ap()
    cos_d = nc.dram_tensor("cosb", [P, S], f32, kind="ExternalInput").ap()
    sin_d = nc.dram_tensor("sinb", [P, S], f32, kind="ExternalInput").ap()
    ones_d = nc.dram_tensor("onesc", [P, 5 * NH], bf16, kind="ExternalInput").ap()
    out_d = nc.dram_tensor("out", [S, D], f32, kind="ExternalOutput").ap()

    with TileContext(nc) as tc, ExitStack() as ctx:
        const = ctx.enter_context(tc.tile_pool(name="const", bufs=1))
        wqkp = ctx.enter_context(tc.tile_pool(name="wqkp", bufs=3))
        wvp = ctx.enter_context(tc.tile_pool(name="wvp", bufs=2))
        wop = ctx.enter_context(tc.tile_pool(name="wop", bufs=2))
        swp = ctx.enter_context(tc.tile_pool(name="swp", bufs=4))
        m2p = ctx.enter_context(tc.tile_pool(name="m2p", bufs=4))
        ep = ctx.enter_context(tc.tile_pool(name="ep", bufs=44))
        r1p = ctx.enter_context(tc.tile_pool(name="r1p", bufs=4))
        rrp = ctx.enter_context(tc.tile_pool(name="rrp", bufs=4))
        stp = ctx.enter_context(tc.tile_pool(name="stp", bufs=2))
        pschain = ctx.enter_context(tc.tile_pool(name="pschain", bufs=3, space="PSUM"))
        pssc = ctx.enter_context(tc.tile_pool(name="pssc", bufs=3, space="PSUM"))
        psav = ctx.enter_context(tc.tile_pool(name="psav", bufs=2, space="PSUM"))

        # ---- resident tensors
        cosb = const.tile([P, S], f32, name="cosb_t")
        sinb = const.tile([P, S], f32, name="sinb_t")
        xt2 = [const.tile([P, NKD * S], bf16, name=f"xt{i}") for i in range(2)]
        roped = const.tile([P, 2 * NH * S], bf16, name="roped")
        va2 = [const.tile([P, 5 * NH * VSLOT], bf16, name=f"va{i}") for i in range(2)]
        vav2 = [va.rearrange("p (j h c) -> p j h c", j=5, c=VSLOT) for va in va2]
        oT = const.tile([P, NKD * S], bf16, name="oT")

        nc.sync.dma_start(cosb[:, :], cos_d[:, :])
        nc.sync.dma_start(sinb[:, :], sin_d[:, :])
        for va in va2:
            nc.sync.dma_start(va.rearrange("p (g c) -> p g c", c=VSLOT)[:, :, HD:HD + 1],
                              ones_d[:, :])

        # weight tiles keyed by (rep, idx) so prefetch and use sites agree
        wqk_tiles: dict = {}
        wv_tiles: dict = {}
        wo_tiles: dict = {}

        def dma_x(r):
            nc.sync.dma_start(xt2[r % 2][:, :], x_d[:, :])

        def dma_wqk(r, g):
            # one DMA covers e-tiles 2g and 2g+1
            wt = wqkp.tile([P, 2 * NKD * P], bf16, name="wt")
            wqk_tiles[(r, g)] = wt
            nc.sync.dma_start(wt[:, :], wqk_d[:, g * 2 * NKD * P:(g + 1) * 2 * NKD * P])

        def dma_wv(r, g):
            wt = wvp.tile([P, NKD * 512], bf16, name="wvt")
            wv_tiles[(r, g)] = wt
            nc.sync.dma_start(wt[:, :], wv_d[:, g * NKD * 512:(g + 1) * NKD * 512])

        def dma_wo(r, g):
            wt = wop.tile([P, NKD * 512], bf16, name="wot")
            wo_tiles[(r, g)] = wt
            nc.sync.dma_start(wt[:, :], wo_d[:, g * NKD * 512:(g + 1) * NKD * 512])

        # ---- EARLY units: v-proj chains and qk-proj + rope -------------------
        def unit_v(r, g, st):
            xt = xt2[r % 2]
            vav = vav2[r % 2]
            s0, sl = S_TILES[st]
            wvt = wv_tiles[(r, g)]
            ps = pschain.tile([P, 512], f32, tag="pc", name="psv")
            for kt in range(NKD):
                nc.tensor.matmul(ps[0:sl, :], xt[:, kt * S + s0:kt * S + s0 + sl],
                                 wvt[:, kt * 512:(kt + 1) * 512],
                                 start=(kt == 0), stop=(kt == NKD - 1))
            dst = vav[0:sl, st:st + 1, g * 8:(g + 1) * 8, 0:HD]
            nc.scalar.activation(dst, ps[0:sl, :].rearrange("p (h cc) -> p h cc", cc=HD),
                                 AF.Copy)

        def unit_qk(r, et, half):
            xt = xt2[r % 2]
            if half == 0:
                if et % 2 == 1 and 2 <= (et + 3) // 2 <= 7:
                    dma_wqk(r, (et + 3) // 2)   # group g at QK(2g-3,0)
                # next-rep prefetches attached at fixed spots in the qk stream
                if r + 1 < repeat:
                    if et == 8:
                        dma_wv(r + 1, 0)
                    elif et == 10:
                        dma_wv(r + 1, 1)
                    elif et == 11:
                        dma_x(r + 1)
                    elif et == 13:
                        dma_wqk(r + 1, 0)
                    elif et == 15:
                        dma_wqk(r + 1, 1)
            wt = wqk_tiles[(r, et // 2)]
            eo = et % 2
            ps = pschain.tile([P, SQH], f32, tag="pc", name="psqk")
            c0 = half * SQH
            for kt in range(NKD):
                nc.tensor.matmul(ps[:, :], wt[:, (eo * NKD + kt) * P:(eo * NKD + kt + 1) * P],
                                 xt[:, kt * S + c0:kt * S + c0 + SQH],
                                 start=(kt == 0), stop=(kt == NKD - 1))
            sw = swp.tile([P, SQH], f32, name="sw")
            nc.vector.stream_shuffle(sw[:, :], ps[:, :], SHUF_MASK)
            m2 = m2p.tile([P, 2 * SQH], f32, name="m2")
            nc.gpsimd.tensor_mul(m2[:, 0:SQH], sw[:, :], sinb[:, c0:c0 + SQH])
            nc.vector.tensor_mul(m2[:, SQH:2 * SQH], ps[:, :], cosb[:, c0:c0 + SQH])
            nc.gpsimd.tensor_add(roped[:, et * S + c0:et * S + c0 + SQH],
                                 m2[:, 0:SQH], m2[:, SQH:2 * SQH])

        # ---- LATE units: scores+exp, att@v+normalize, out-proj ---------------
        def unit_sc(r, ti, hf, jg):
            if hf == 0 and jg == 0 and ti in (3, 4):
                dma_wo(r, ti - 3)
            qb = ti * S
            kb = (8 + ti) * S
            col = slice(qb + hf * SQH, qb + (hf + 1) * SQH)
            js = [(0, 1), (2, 3), (4,)][jg]
            out = []
            for j in js:
                k0, kl = S_TILES[j]
                for sub in range(2):
                    off = sub * 64
                    ps = pssc.tile([P, SQH], f32, tag="sc", name="ps_s")
                    nc.tensor.matmul(ps[0:kl, :],
                                     roped[off:off + 64, kb + k0:kb + k0 + kl],
                                     roped[off:off + 64, col],
                                     start=True, stop=True)
                    E = ep.tile([P, SQH], bf16, name="E")
                    nc.scalar.activation(E[0:kl, :], ps[0:kl, :], AF.Exp, scale=0.125)
                    out.append((j, sub, E))
            return out

        def unit_av(r, ti, hf, Es):
            vav = vav2[r % 2]
            for sub in range(2):
                h = 2 * ti + sub
                off = sub * 64
                ps_o = psav.tile([P, SQH], f32, tag="av", name="ps_o")
                for j, (k0, kl) in enumerate(S_TILES):
                    nc.tensor.matmul(ps_o[0:VSLOT, :],
                                     vav[0:kl, j:j + 1, h:h + 1, :],
                                     Es[(ti, hf, j, sub)][0:kl, :],
                                     start=(j == 0), stop=(j == 4))
                r1 = r1p.tile([1, SQH], f32, name="r1")
                nc.vector.reciprocal(r1[:, :], ps_o[HD:HD + 1, :])
                rr = rrp.tile([64, SQH], f32, name="rr")
                nc.gpsimd.partition_broadcast(rr[:, :], r1[:, :])
                dst = oT[off:off + 64, ti * S + hf * SQH:ti * S + (hf + 1) * SQH]
                nc.vector.tensor_mul(dst, ps_o[0:HD, :], rr[:, :])

        def unit_op(r, st):
            s0, sl = S_TILES[st]
            stg = stp.tile([P, D], f32, name="stg")
            for g in range(2):
                wot = wo_tiles[(r, g)]
                ps = pschain.tile([P, 512], f32, tag="pc", name="pso")
                for kt in range(NKD):
                    nc.tensor.matmul(ps[0:sl, :], oT[:, kt * S + s0:kt * S + s0 + sl],
                                     wot[:, kt * 512:(kt + 1) * 512],
                                     start=(kt == 0), stop=(kt == NKD - 1))
                nc.vector.tensor_copy(stg[0:sl, g * 512:(g + 1) * 512], ps[0:sl, :])
            nc.sync.dma_start(out_d[s0:s0 + sl, :], stg[0:sl, :])

        # ---- emission schedule ----------------------------------------------
        def early_units(r):
            units = []
            for g in range(2):
                for st in range(5):
                    units.append(lambda r=r, g=g, st=st: unit_v(r, g, st))
            for et in range(16):
                for half in range(2):
                    units.append(lambda r=r, et=et, half=half: unit_qk(r, et, half))
            return units

        def late_units(r):
            Es: dict = {}

            def sc(ti, hf, jg):
                for j, sub, E in unit_sc(r, ti, hf, jg):
                    Es[(ti, hf, j, sub)] = E

            def sc_block(ti):
                return [lambda ti=ti, hf=hf, jg=jg: sc(ti, hf, jg)
                        for hf in range(2) for jg in range(3)]

            # Es is mutated as sc units run; av reads it lazily at call time.
            def av_unit(ti, hf):
                return lambda: unit_av(r, ti, hf, Es)

            # SC(0), SC(1), AV(0), SC(2), AV(1), ..., SC(7), AV(6), AV(7)
            units = []
            units += sc_block(0)
            units += sc_block(1)
            units += [av_unit(0, 0), av_unit(0, 1)]
            for ti in range(2, 8):
                units += sc_block(ti)
                units += [av_unit(ti - 1, 0), av_unit(ti - 1, 1)]
            units += [av_unit(7, 0), av_unit(7, 1)]
            for st in range(5):
                units.append(lambda st=st: unit_op(r, st))
            return units

        def merged_emit(late, early):
            nl, ne = len(late), len(early)
            li = ei = 0
            while li < nl or ei < ne:
                if li < nl and (ei >= ne or li * ne <= ei * nl):
                    late[li]()
                    li += 1
                else:
                    early[ei]()
                    ei += 1

        # prologue DMAs for rep 0
        dma_x(0)
        dma_wv(0, 0)
        dma_wv(0, 1)
        dma_wqk(0, 0)
        dma_wqk(0, 1)

        for r in range(repeat):
            merged_emit(late_units(r - 1) if r > 0 else [], early_units(r))
        merged_emit(late_units(repeat - 1), [])

    nc.compile()
    return nc


def _prep_inputs(x, w_qkv, w_out):
    x = np.asarray(x, dtype=np.float32)
    w_qkv = np.asarray(w_qkv, dtype=np.float32)
    w_out = np.asarray(w_out, dtype=np.float32)
    xr = x.reshape(BT, S, D)
    perm = _head_perm()
    wq = w_qkv[0:D][perm]
    wk = w_qkv[D:2 * D][perm]
    wqkT = np.concatenate([wq, wk], axis=0).T          # [1024 d, 2048 e]
    # pack: [d=kt*128+p, e=et*128+c] -> [p, et*1024 + kt*128 + c]
    wqk_pack = np.ascontiguousarray(
        wqkT.reshape(NKD, P, 16, P).transpose(1, 2, 0, 3).reshape(P, 16 * NKD * P)
        .astype(_bf16_np()))
    wvT = w_qkv[2 * D:3 * D].T                         # [1024 d, 1024 e]
    wv_pack = np.ascontiguousarray(
        wvT.reshape(NKD, P, 2, 512).transpose(1, 2, 0, 3).reshape(P, 2 * NKD * 512)
        .astype(_bf16_np()))
    woT = w_out.T
    wo_pack = np.ascontiguousarray(
        woT.reshape(NKD, P, 2, 512).transpose(1, 2, 0, 3).reshape(P, 2 * NKD * 512)
        .astype(_bf16_np()))
    cosb, sinb = _rope_tables()
    in_maps = []
    for i in range(NCORES):
        xp = np.ascontiguousarray(
            xr[i].T.reshape(NKD, P, S).transpose(1, 0, 2).reshape(P, NKD * S)
            .astype(_bf16_np()))
        in_maps.append({
            "xp": xp,
            "wqkp": wqk_pack, "wvp": wv_pack, "wop": wo_pack,
            "cosb": cosb, "sinb": sinb,
            "onesc": np.ones((P, 5 * NH), dtype=_bf16_np()),
        })
    return in_maps


def get_nc(repeat=1):
    key = f"nc{repeat}"
    if key not in _CACHE:
        _CACHE[key] = _build_nc(repeat)
    return _CACHE[key]


def kernel(x, w_qkv, w_out, b_out):
    from concourse import bass_utils
    nc = get_nc()
    in_maps = _prep_inputs(x, w_qkv, w_out)
    res = bass_utils.run_bass_kernel_spmd(nc, in_maps, core_ids=list(range(NCORES)))
    out = np.stack([res.results[i]["out"] for i in range(NCORES)], axis=0)
    out = out + np.asarray(b_out, dtype=np.float32)[None, None, :]
    return np.ascontiguousarray(out.reshape(B, T, HH, WW, D).astype(np.float32))
